# revision 26
# baseline (speedup 1.0000x reference)
"""Trainium2 Bass kernel for ConvTemporalGraphical-style gated graph conv.

Computation (see reference):
    g   = x.reshape(N, F)                       # F = C*T*V = 204800
    h0  = elu(g @ W0 + b0)                      # [N, 256]   <-- dominant cost
    h1  = elu(h0 @ W1 + b1)                     # [N, 256]
    w   = softmax(h1 @ W2 + b2)                 # [N, 4]
    AS  = einsum('ne,etvw->ntvw', w, A)         # [N, T, V, V]
    out = einsum('nctv,ntvw->nctw', x, AS)

Sharding across 8 NeuronCores (one chip):
  * The F (contraction) dim of the big gating matmul is split 8 ways:
    core c holds W0 rows [c*25600, (c+1)*25600) (26 MB instead of 210 MB)
    and the matching slice of x, producing a partial h0 [32, 256].
  * A tiny AllReduce (32 KB) combines the partials; every core then runs the
    small MLP + softmax redundantly for all 32 samples.
  * The mixture + graph conv is data-parallel: core c owns samples
    [4c, 4c+4), selected on-device via a per-core one-hot matrix so all
    cores run the same graph (SPMD).

Layout tricks:
  * x slices are transposed on the TensorEngine (fp32 has no DMA transpose)
    via identity-matmul into PSUM, batched 8 per PSUM bank.
  * The graph conv runs as 25x25(x64) matmuls packed 4-at-a-time into the
    128x128 PE array with `tile_position` row groups; x^T tiles are produced
    directly in the 32-aligned padded layout by transposing with a gapped
    access pattern, and A is DMA'd into the same padded layout.
  * The gating matmul runs in float32r (full-rate PE mode; the 256-wide
    moving operand qualifies) with fp32 PSUM accumulation.
"""

import sys

if "/opt/trn_rl_repo" not in sys.path:
    sys.path.insert(0, "/opt/trn_rl_repo")

import numpy as np

import concourse.bass as bass
import concourse.mybir as mybir
import concourse.tile as tile
from concourse import bacc
from concourse import bass_utils
from concourse.masks import make_identity

# Problem dims (hardcoded per contract).
N, C, T, V = 32, 64, 128, 25
F = C * T * V            # 204800
H = 256
E = 4
NCORES = 8
KS = F // NCORES         # 25600 rows of W0 per core
NCH = C // NCORES        # 8 channels of x per core (gating slice)
NLOC = N // NCORES       # 4 samples per core (conv slice)
KCH = KS // 128          # 200 k-chunks of 128 per core
GGRP = 8                 # k-chunks per transpose/DMA group
NGRP = KCH // GGRP       # 25 groups
TG = T // 4              # 32 groups of 4 t's for the conv
CONV_G_PER_BANK = 4      # conv psum tile covers 4 t-groups (16 t, 400 cols)

FP32 = mybir.dt.float32
F32R = mybir.dt.float32r
AX = mybir.AxisListType
ALU = mybir.AluOpType
ACTF = mybir.ActivationFunctionType

# Knobs.
CFG = {
    "gating_dtype": "bf16",   # "bf16" | "f32"
    "conv_dtype": "f32",
    "phase": 5,               # 1..5 truncation for HW bisection
}


def _gdt():
    return mybir.dt.bfloat16 if CFG["gating_dtype"] == "bf16" else FP32


def build():
    nc = bacc.Bacc("TRN2", target_bir_lowering=False, debug=False, num_devices=NCORES)

    xg = nc.dram_tensor("xg", [N, KS], FP32, kind="ExternalInput")
    xc = nc.dram_tensor("xc", [NLOC, C, T * V], FP32, kind="ExternalInput")
    W0s = nc.dram_tensor("W0s", [KS, H], FP32, kind="ExternalInput")
    b0 = nc.dram_tensor("b0", [H], FP32, kind="ExternalInput")
    W1 = nc.dram_tensor("W1", [H, H], FP32, kind="ExternalInput")
    b1 = nc.dram_tensor("b1", [H], FP32, kind="ExternalInput")
    W2 = nc.dram_tensor("W2", [H, E], FP32, kind="ExternalInput")
    b2 = nc.dram_tensor("b2", [E], FP32, kind="ExternalInput")
    A4 = nc.dram_tensor("A4", [E, T, V * V], FP32, kind="ExternalInput")
    selT = nc.dram_tensor("selT", [N, NLOC], FP32, kind="ExternalInput")
    out = nc.dram_tensor("out", [NLOC, C, T * V], FP32, kind="ExternalOutput")

    with tile.TileContext(nc) as tc:
        _build_body(nc, tc, xg, xc, W0s, b0, W1, b1, W2, b2, A4, selT, out)
    nc.compile()
    return nc


def _build_body(nc, tc, xg, xc, W0s, b0, W1, b1, W2, b2, A4, selT, out):
    from contextlib import ExitStack

    # Accept either tensor handles or APs (run_kernel passes APs).
    def _as_ap(t):
        return t if isinstance(t, bass.AP) else t.ap()

    xg, xc, W0s, b0, W1, b1, W2, b2, A4, selT, out = map(
        _as_ap, (xg, xc, W0s, b0, W1, b1, W2, b2, A4, selT, out)
    )

    ctx = ExitStack()
    with ctx:
        const = ctx.enter_context(tc.tile_pool(name="const", bufs=1))
        xg_pool = ctx.enter_context(tc.tile_pool(name="xg_pool", bufs=3))
        w0_pool = ctx.enter_context(tc.tile_pool(name="w0_pool", bufs=3))
        mix_pool = ctx.enter_context(tc.tile_pool(name="mix_pool", bufs=2))
        xc_pool = ctx.enter_context(tc.tile_pool(name="xc_pool", bufs=2))
        out_pool = ctx.enter_context(tc.tile_pool(name="out_pool", bufs=3))
        dram = ctx.enter_context(tc.tile_pool(name="dram", bufs=1, space="DRAM"))
        # PSUM pools: 8 banks total.
        # PSUM bank budget (8 total): pg 1 + pt 2 + ph 1 + pc (4 tags x 1) 4
        pg = ctx.enter_context(tc.tile_pool(name="pg", bufs=1, space="PSUM"))
        pt = ctx.enter_context(tc.tile_pool(name="pt", bufs=2, space="PSUM"))
        ph = ctx.enter_context(tc.tile_pool(name="ph", bufs=1, space="PSUM"))
        pc = ctx.enter_context(tc.tile_pool(name="pc", bufs=1, space="PSUM"))

        # ---- constants ----
        identity = const.tile([128, 128], FP32)
        make_identity(nc, identity)

        b0_row = const.tile([1, H], FP32)
        nc.sync.dma_start(b0_row[:], b0.rearrange("(o h) -> o h", o=1))
        b0b = const.tile([N, H], FP32)
        nc.gpsimd.partition_broadcast(b0b[:], b0_row[:])

        b1_row = const.tile([1, H], FP32)
        nc.sync.dma_start(b1_row[:], b1.rearrange("(o h) -> o h", o=1))
        b1b = const.tile([N, H], FP32)
        nc.gpsimd.partition_broadcast(b1b[:], b1_row[:])

        b2_row = const.tile([1, E], FP32)
        nc.sync.dma_start(b2_row[:], b2.rearrange("(o h) -> o h", o=1))
        b2b = const.tile([N, E], FP32)
        nc.gpsimd.partition_broadcast(b2b[:], b2_row[:])

        W1_sb = const.tile([128, 2, H], FP32)
        nc.sync.dma_start(W1_sb[:], W1.rearrange("(j p) h -> p j h", p=128))
        W2_sb = const.tile([128, 2, E], FP32)
        nc.sync.dma_start(W2_sb[:], W2.rearrange("(j p) h -> p j h", p=128))
        selT_sb = const.tile([N, NLOC], FP32)
        nc.sync.dma_start(selT_sb[:], selT[:])

        # ---- persistent big SBUF tensors ----
        xT_all = const.tile([128, KCH, N], _gdt())        # gating x^T chunks
        xcT_all = const.tile([128, NLOC, TG, C], FP32)    # conv x^T (padded rows)
        A_sb = const.tile([128, E, TG * V], FP32)         # A in (b,v) x (g,w) layout
        AS_sb = const.tile([128, NLOC, TG * V], FP32)     # mixture output

        # =========================================================
        # Phase A1: gating x slice -> SBUF -> PE transpose -> xT_all
        # =========================================================
        for g in range(NGRP):
            xg_t = xg_pool.tile([N, GGRP * 128], FP32, tag="xg_t")
            nc.sync.dma_start(xg_t[:], xg[:, g * GGRP * 128:(g + 1) * GGRP * 128])
            ps_t = pt.tile([128, GGRP * N], FP32, tag="ps_tc")
            for j in range(GGRP):
                nc.tensor.transpose(
                    ps_t[:, j * N:(j + 1) * N],
                    xg_t[:, j * 128:(j + 1) * 128],
                    identity[:N, :N],
                )
            nc.vector.tensor_copy(
                xT_all[:, g * GGRP:(g + 1) * GGRP, :].rearrange("p j n -> p (j n)"),
                ps_t[:],
            )

        # =========================================================
        # Phase A2: W0 slice load + gating matmul accumulation
        # =========================================================
        h0_ps = pg.tile([N, H], FP32)
        gdt = _gdt()
        for g in range(NGRP):
            w0_t = w0_pool.tile([128, GGRP, H], gdt, tag="w0_t")
            w0_src = W0s.rearrange("(g j p) h -> g p j h", j=GGRP, p=128)[g]
            if gdt == FP32:
                nc.sync.dma_start(w0_t[:], w0_src)
            else:
                # SWDGE cast-during-DMA fp32 -> bf16
                nc.gpsimd.dma_start(w0_t[:], w0_src)
            for j in range(GGRP):
                k = g * GGRP + j
                nc.tensor.matmul(
                    h0_ps[:],
                    xT_all[:, k, :],
                    w0_t[:, j, :],
                    start=(k == 0),
                    stop=(k == KCH - 1),
                )

        if CFG["phase"] == 1:
            p1 = const.tile([N, H], FP32)
            nc.vector.tensor_copy(p1[:], h0_ps[:])
            nc.sync.dma_start(out[0][:N, :H], p1[:])
            return

        # =========================================================
        # Phase B (overlaps A): conv-side loads and transposes
        # =========================================================
        # x conv slice: DMA into a v-padded layout (col = g*128 + b*32 + v) so
        # each PE-transpose input is a contiguous [64, 128] slice and its
        # output lands on 32-aligned partition blocks (t = 4g + b).
        for n in range(NLOC if CFG["phase"] >= 4 else 0):
            xc_t = xc_pool.tile([C, TG * 128], FP32, tag="xc_t")
            # zero the pad columns (vp 25..31 of every 32-block)
            nc.vector.memset(
                xc_t.rearrange("c (g b vp) -> c g b vp", b=4, vp=32)[:, :, :, V:],
                0.0,
            )
            for b in range(4):
                nc.sync.dma_start(
                    xc_t.rearrange("c (g b vp) -> c g b vp", b=4, vp=32)[:, :, b, :V],
                    xc[n][:, (32 * b) * V:(32 * b + 32) * V].rearrange(
                        "c (g v) -> c g v", v=V
                    ),
                )
            for gq in range(0, TG, GGRP):
                ps_c = pt.tile([128, GGRP * C], FP32, tag="ps_tc")
                for gi in range(GGRP):
                    g = gq + gi
                    nc.tensor.transpose(
                        ps_c[:, gi * C:(gi + 1) * C],
                        xc_t[:, g * 128:(g + 1) * 128],
                        identity[:C, :C],
                    )
                nc.vector.tensor_copy(
                    xcT_all[:, n, gq:gq + GGRP, :].rearrange("p g c -> p (g c)"),
                    ps_c[:],
                )

        # A -> padded SBUF layout: partition 32*b + v, free (g, w).
        # zero A_sb first so the 7 pad rows of each 32-row block read 0.0
        if CFG["phase"] >= 4:
            nc.gpsimd.memset(A_sb[:], 0.0)
        for e in range(E if CFG["phase"] >= 4 else 0):
            for b in range(4):
                nc.sync.dma_start(
                    A_sb[32 * b:32 * b + V, e, :].rearrange("v (g w) -> v g w", w=V),
                    A4[e].rearrange("(b g) (v w) -> b v g w", b=4, w=V)[b],
                )

        # =========================================================
        # Phase C: partial-h0 AllReduce (tiny; runs on TOPSP/SDMA)
        # =========================================================
        h0p_sb = const.tile([N, H], FP32)
        nc.vector.tensor_copy(h0p_sb[:], h0_ps[:])
        cc_in = dram.tile([N, H], FP32)
        cc_out = dram.tile([N, H], FP32, addr_space="Shared")
        nc.gpsimd.dma_start(cc_in[:], h0p_sb[:])
        nc.gpsimd.collective_compute(
            "AllReduce",
            ALU.add,
            replica_groups=[list(range(NCORES))],
            ins=[cc_in.opt()],
            outs=[cc_out.opt()],
        )
        h0_sb = const.tile([N, H], FP32)
        nc.sync.dma_start(h0_sb[:], cc_out[:])
        if CFG["phase"] == 2:
            nc.sync.dma_start(out[0][:N, :H], h0_sb[:])
            return

        # =========================================================
        # Phase D: tiny MLP + softmax + local-w selection/broadcast
        # =========================================================
        def elu_inplace(t, tmp_pool, width):
            tmp = tmp_pool.tile([N, width], FP32, tag="elu_tmp")
            nc.vector.tensor_scalar(tmp[:], t[:], 0.0, None, ALU.min)
            nc.scalar.activation(tmp[:], tmp[:], ACTF.Exp)
            nc.vector.tensor_scalar(t[:], t[:], 0.0, -1.0, ALU.max, ALU.add)
            nc.vector.tensor_tensor(t[:], t[:], tmp[:], ALU.add)

        nc.vector.tensor_tensor(h0_sb[:], h0_sb[:], b0b[:], ALU.add)
        elu_inplace(h0_sb, const, H)

        # h0^T
        ps_h = ph.tile([128, 2 * N], FP32, tag="mlp_ps")
        for j in range(2):
            nc.tensor.transpose(
                ps_h[:, j * N:(j + 1) * N],
                h0_sb[:, j * 128:(j + 1) * 128],
                identity[:N, :N],
            )
        h0T = const.tile([128, 2, N], FP32)
        nc.vector.tensor_copy(h0T[:].rearrange("p j n -> p (j n)"), ps_h[:])

        h1_ps = ph.tile([N, H], FP32, tag="mlp_ps")
        for j in range(2):
            nc.tensor.matmul(
                h1_ps[:], h0T[:, j, :], W1_sb[:, j, :],
                start=(j == 0), stop=(j == 1),
            )
        h1_sb = const.tile([N, H], FP32)
        nc.vector.tensor_copy(h1_sb[:], h1_ps[:])
        nc.vector.tensor_tensor(h1_sb[:], h1_sb[:], b1b[:], ALU.add)
        elu_inplace(h1_sb, const, H)

        ps_h2 = ph.tile([128, 2 * N], FP32, tag="mlp_ps")
        for j in range(2):
            nc.tensor.transpose(
                ps_h2[:, j * N:(j + 1) * N],
                h1_sb[:, j * 128:(j + 1) * 128],
                identity[:N, :N],
            )
        h1T = const.tile([128, 2, N], FP32)
        nc.vector.tensor_copy(h1T[:].rearrange("p j n -> p (j n)"), ps_h2[:])

        lg_ps = ph.tile([N, E], FP32, tag="mlp_ps")
        for j in range(2):
            nc.tensor.matmul(
                lg_ps[:], h1T[:, j, :], W2_sb[:, j, :],
                start=(j == 0), stop=(j == 1),
            )
        lg_sb = const.tile([N, E], FP32)
        nc.vector.tensor_copy(lg_sb[:], lg_ps[:])
        nc.vector.tensor_tensor(lg_sb[:], lg_sb[:], b2b[:], ALU.add)

        # softmax over E (free dim)
        mx = const.tile([N, 1], FP32)
        nc.vector.reduce_max(mx[:], lg_sb[:], axis=AX.X)
        negmx = const.tile([N, 1], FP32)
        nc.vector.tensor_scalar_mul(negmx[:], mx[:], -1.0)
        ex = const.tile([N, E], FP32)
        sm = const.tile([N, 1], FP32)
        nc.scalar.activation(ex[:], lg_sb[:], ACTF.Exp, bias=negmx[:], accum_out=sm[:])
        rec = const.tile([N, 1], FP32)
        nc.vector.reciprocal(rec[:], sm[:])
        w_sb = const.tile([N, E], FP32)
        nc.vector.tensor_scalar(w_sb[:], ex[:], rec[:], None, ALU.mult)

        # local w: [4, 4] = selT^T @ w  (K=32)
        wl_ps = ph.tile([NLOC, E], FP32, tag="mlp_ps")
        nc.tensor.matmul(wl_ps[:], selT_sb[:], w_sb[:], start=True, stop=True)
        wloc = const.tile([NLOC, E], FP32)
        nc.vector.tensor_copy(wloc[:], wl_ps[:])

        # flatten [4, 4] -> [1, 16] with a partition-crossing SBUF DMA, then
        # broadcast partition 0 to all 128 partitions.
        w_row = const.tile([1, NLOC * E], FP32)
        nc.gpsimd.dma_start(
            w_row.rearrange("o (n e) -> o n e", n=NLOC), wloc[:]
        )
        w_bcast = const.tile([128, NLOC * E], FP32)
        nc.gpsimd.partition_broadcast(w_bcast[:], w_row[:])
        if CFG["phase"] == 3:
            nc.sync.dma_start(out[0][:N, 0:E], lg_sb[:])
            nc.sync.dma_start(out[0][:N, 100:100 + E], w_sb[:])
            nc.sync.dma_start(out[0][:NLOC, 200:200 + E], wloc[:])
            nc.sync.dma_start(out[0][:1, 300:300 + NLOC * E], w_row[:])
            nc.sync.dma_start(out[0][:, 400:400 + NLOC * E], w_bcast[:C, :])
            nc.sync.dma_start(out[0][:N, 500:500 + H], h1_sb[:])
            nc.sync.dma_start(out[0][:N, 800:800 + H], h0_sb[:])
            nc.sync.dma_start(out[0][:N, 1100:1100 + 1], mx[:])
            nc.sync.dma_start(out[0][:N, 1200:1200 + E], ex[:])
            nc.sync.dma_start(out[0][:N, 1300:1300 + 1], sm[:])
            return

        # =========================================================
        # Phase E: mixture AS = sum_e w[n,e] * A[e]  (padded layout)
        # =========================================================
        for n in range(NLOC):
            acc = mix_pool.tile([128, TG * V], FP32, tag="mix_acc")
            tmp = mix_pool.tile([128, TG * V], FP32, tag="mix_tmp")
            nc.scalar.activation(
                acc[:], A_sb[:, 0, :], ACTF.Copy, scale=w_bcast[:, n * E:n * E + 1]
            )
            nc.vector.tensor_scalar(
                tmp[:], A_sb[:, 1, :], w_bcast[:, n * E + 1:n * E + 2], None, ALU.mult
            )
            nc.vector.tensor_tensor(acc[:], acc[:], tmp[:], ALU.add)
            nc.scalar.activation(
                tmp[:], A_sb[:, 2, :], ACTF.Copy, scale=w_bcast[:, n * E + 2:n * E + 3]
            )
            nc.vector.tensor_tensor(acc[:], acc[:], tmp[:], ALU.add)
            nc.vector.tensor_scalar(
                tmp[:], A_sb[:, 3, :], w_bcast[:, n * E + 3:n * E + 4], None, ALU.mult
            )
            nc.vector.tensor_tensor(
                AS_sb[:, n, :], acc[:], tmp[:], ALU.add
            )
        if CFG["phase"] == 4:
            nc.sync.dma_start(out[0][:, :TG * V], AS_sb[:C, 0, :])
            return

        # =========================================================
        # Phase F: graph conv, 4-way row-packed 25x25x64 matmuls.
        # t = 32*b + g: row-block b owns a contiguous quarter of the time
        # axis and writes its OWN psum bank (different row tiles must not
        # touch the same PSUM bank concurrently), and each bank drains to a
        # contiguous column range of out[n].
        # =========================================================
        for n in range(NLOC):
            for g0, glen in ((0, 20), (20, 12)):
                pob = [
                    pc.tile([C, 20 * V], FP32, tag=f"po{b}", name=f"po{b}")
                    for b in range(4)
                ]
                for gi in range(glen):
                    g = g0 + gi
                    for b in range(4):
                        nc.tensor.matmul(
                            pob[b][:, gi * V:(gi + 1) * V],
                            xcT_all[32 * b:32 * b + V, n, g, :],
                            AS_sb[32 * b:32 * b + V, n, g * V:(g + 1) * V],
                            start=True,
                            stop=True,
                            tile_position=(32 * b, 0),
                        )
                width = glen * V
                for b in range(4):
                    ot = out_pool.tile([C, 20 * V], FP32, tag="ot")
                    nc.vector.tensor_copy(ot[:, :width], pob[b][:, :width])
                    nc.sync.dma_start(
                        out[n][:, (32 * b + g0) * V:(32 * b + g0) * V + width],
                        ot[:, :width],
                    )


_NC_CACHE = {}


def _get_nc():
    key = (CFG["gating_dtype"], CFG["conv_dtype"], CFG["phase"])
    if key not in _NC_CACHE:
        _NC_CACHE[key] = build()
    return _NC_CACHE[key]


def _shard_inputs(x, W0, b0, W1, b1, W2, b2, A):
    x = np.ascontiguousarray(np.asarray(x, dtype=np.float32))
    W0 = np.ascontiguousarray(np.asarray(W0, dtype=np.float32))
    xf = x.reshape(N, F)
    in_maps = []
    for c in range(NCORES):
        sel = np.zeros((N, NLOC), dtype=np.float32)
        for i in range(NLOC):
            sel[c * NLOC + i, i] = 1.0
        in_maps.append({
            "xg": np.ascontiguousarray(xf[:, c * KS:(c + 1) * KS]),
            "xc": np.ascontiguousarray(
                x[c * NLOC:(c + 1) * NLOC].reshape(NLOC, C, T * V)
            ),
            "W0s": np.ascontiguousarray(W0[c * KS:(c + 1) * KS]),
            "b0": np.asarray(b0, dtype=np.float32),
            "W1": np.asarray(W1, dtype=np.float32),
            "b1": np.asarray(b1, dtype=np.float32),
            "W2": np.asarray(W2, dtype=np.float32),
            "b2": np.asarray(b2, dtype=np.float32),
            "A4": np.ascontiguousarray(
                np.asarray(A, dtype=np.float32).reshape(E, T, V * V)
            ),
            "selT": sel,
        })
    return in_maps


def kernel(x, W0, b0, W1, b1, W2, b2, A):
    nc = _get_nc()
    in_maps = _shard_inputs(x, W0, b0, W1, b1, W2, b2, A)
    res = bass_utils.run_bass_kernel_spmd(nc, in_maps, core_ids=list(range(NCORES)))
    outs = [res.results[c]["out"].reshape(NLOC, C, T, V) for c in range(NCORES)]
    return np.concatenate(outs, axis=0)
